# revision 11
# baseline (speedup 1.0000x reference)
"""Trainium2 Bass kernel for nn_AdditiveAttention (B=8, S=4096, D=1024, H=16).

Sharding: pure data-parallel over batch — 8 NeuronCores, one batch element
per core, weights replicated. No collectives.

v2 layout (everything transposed: d on partitions, s on free):
  - Q GEMM is n-outer (s-chunk outer, t-tile inner) so each xt s-chunk is
    dead right after its 8 output tiles are computed; q (bf16, +bq+br folded
    in) is written back into the xt chunk buffers with a one-chunk lag.
    Only one spare [128, 8, 512] buffer is needed for q chunk 0.
  - Per-chunk softmax pooling: logit matmul emitted one GEMM-slot late so
    PSUM evacuation always overlaps the next chunk's matmuls; exp+denominator
    fused on ScalarE (accum_out); numerator via one fused DVE
    tensor_tensor_reduce per chunk. No full-S e/p/u tiles anywhere.
  - K/V GEMMs in fp8 DoubleRow (weights host-scaled by 64; the 1/64 rides in
    the pooled-vector normalization), per-chunk gated logits / rt matmuls
    pipelined the same way.
  - Single bf16 output: out = q + (u @ Wr) (+bq+br already inside q),
    residual added during rt-PSUM evacuation on DVE. 8.4 MB written instead
    of the baseline's 33.6 MB f32 out+out2 pair.
  - wk/wv/xt8 prefetched on the scalar queue during the Q phase; startup
    loads are chunk-granular ([128,512]) and ordered so chunk 0 + wq arrive
    first on two issuing queues.
"""

import sys
import types

import numpy as np
import ml_dtypes

from contextlib import ExitStack

import concourse.bass as bass
import concourse.tile as tile
from concourse import bacc, mybir
from concourse.bass_utils import run_bass_kernel_spmd

B, S, D, H, HD = 8, 4096, 1024, 16, 64
P = 128          # partitions
T = D // P       # 8 d-tiles
NC_ = 512        # psum chunk free size
NS = S // NC_    # 8 s-chunks
N_CORES = 8
BF16 = mybir.dt.bfloat16
FP8 = mybir.dt.float8e4
F32 = mybir.dt.float32
W8SCALE = 64.0   # host scales Wk/Wv by this into e4m3 normal range
INV_W8 = 1.0 / W8SCALE
BF = ml_dtypes.bfloat16
F8 = ml_dtypes.float8_e4m3
OUT_DT = BF16  # bf16 halves output DMA traffic; host upcasts
USE_TTR = False  # fused DVE multiply+reduce for the pooled sums

_CACHE = {}


def _build():
    nc = bacc.Bacc(
        "TRN2", target_bir_lowering=False, debug=False, num_devices=N_CORES
    )
    xt_ext = nc.declare_dram_parameter("xt", [D, S], BF16, isOutput=False)
    xt8_ext = nc.declare_dram_parameter("xt8", [D, S], FP8, isOutput=False)
    wq_ext = nc.declare_dram_parameter("wq", [D, D], BF16, isOutput=False)
    wk_ext = nc.declare_dram_parameter("wk", [D, D], FP8, isOutput=False)
    wv_ext = nc.declare_dram_parameter("wv", [D, D], FP8, isOutput=False)
    bqbr_ext = nc.declare_dram_parameter("bqbr", [P, T], F32, isOutput=False)
    bk_ext = nc.declare_dram_parameter("bk64", [P, T], F32, isOutput=False)
    bv_ext = nc.declare_dram_parameter("bv64", [P, T], F32, isOutput=False)
    br64_ext = nc.declare_dram_parameter("br64", [P, 1], F32, isOutput=False)
    wql_ext = nc.declare_dram_parameter("wqlrep", [P, P], BF16, isOutput=False)
    wkl_ext = nc.declare_dram_parameter("wklrep", [P, P], BF16, isOutput=False)
    wrr_ext = nc.declare_dram_parameter("wrr", [P, P], BF16, isOutput=False)
    out_ext = nc.declare_dram_parameter("out", [D, S], OUT_DT, isOutput=True)

    AX = mybir.AxisListType.X
    ALU = mybir.AluOpType
    AF = mybir.ActivationFunctionType
    DR = mybir.MatmulPerfMode.DoubleRow

    with tile.TileContext(nc) as tc, ExitStack() as ctx:
        singles = ctx.enter_context(tc.tile_pool(name="singles", bufs=1))
        psg = ctx.enter_context(tc.tile_pool(name="psg", bufs=3, space="PSUM"))
        psl = ctx.enter_context(tc.tile_pool(name="psl", bufs=3, space="PSUM"))
        chk_pool = ctx.enter_context(tc.tile_pool(name="chk", bufs=3))
        e_pool = ctx.enter_context(tc.tile_pool(name="epool", bufs=2))
        m_pool = ctx.enter_context(tc.tile_pool(name="mpool", bufs=2))
        stg_pool = ctx.enter_context(tc.tile_pool(name="stg", bufs=3))
        eff_pool = ctx.enter_context(tc.tile_pool(name="eff", bufs=2))
        small_pool = ctx.enter_context(tc.tile_pool(name="small", bufs=2))

        # ---- resident tiles ----
        xtc = [
            singles.tile([P, T, NC_], BF16, name=f"xtc{n}", tag=f"xtc{n}")
            for n in range(NS)
        ]
        qsp = singles.tile([P, T, NC_], BF16, name="qsp", tag="qsp")
        xt8 = singles.tile([P, T, S], FP8, name="xt8", tag="xt8")
        wq = singles.tile([P, T, D], BF16, name="wq", tag="wq")
        wk = singles.tile([P, T, D], FP8, name="wk", tag="wk")
        wv = singles.tile([P, T, D], FP8, name="wv", tag="wv")
        wqlrep = singles.tile([P, P], BF16, name="wqlrep", tag="wqlrep")
        wklrep = singles.tile([P, P], BF16, name="wklrep", tag="wklrep")
        wrr = singles.tile([P, P], BF16, name="wrr", tag="wrr")
        bqbr = singles.tile([P, T], F32, name="bqbr", tag="bqbr")
        bk64 = singles.tile([P, T], F32, name="bk64", tag="bk64")
        bv64 = singles.tile([P, T], F32, name="bv64", tag="bv64")
        br64 = singles.tile([P, 1], F32, name="br64", tag="br64")
        gq_all = singles.tile([P, T], F32, name="gq", tag="gq")
        gk_all = singles.tile([P, T], F32, name="gk", tag="gk")
        pe_q = singles.tile([P, T * NS], F32, name="peq", tag="peq")
        pq_q = singles.tile([P, T * NS], F32, name="pqq", tag="pqq")
        pe_k = singles.tile([P, T * NS], F32, name="pek", tag="pek")
        pq_k = singles.tile([P, T * NS], F32, name="pqk", tag="pqk")

        # ---- DMA issue (ordering matters for startup) ----
        # sync: xt chunks 0,2,4,6 (chunk 0 first: the first GEMM chunk).
        for n in (0, 2, 4, 6):
            for k in range(T):
                nc.sync.dma_start(
                    xtc[n][:, k, :],
                    xt_ext.ap()[k * P : (k + 1) * P, n * NC_ : (n + 1) * NC_],
                )
        # gpsimd: wq first (needed by every chunk), then odd xt chunks.
        for k in range(T):
            nc.gpsimd.dma_start(wq[:, k, :], wq_ext.ap()[k * P : (k + 1) * P, :])
        for n in (1, 3, 5, 7):
            for k in range(T):
                nc.gpsimd.dma_start(
                    xtc[n][:, k, :],
                    xt_ext.ap()[k * P : (k + 1) * P, n * NC_ : (n + 1) * NC_],
                )
        # scalar: small weights/biases, then K/V weights + fp8 X (all needed
        # only from the K phase on).
        nc.scalar.dma_start(wqlrep[:], wql_ext.ap())
        nc.scalar.dma_start(bqbr[:], bqbr_ext.ap())
        nc.scalar.dma_start(br64[:], br64_ext.ap())
        nc.scalar.dma_start(wklrep[:], wkl_ext.ap())
        nc.scalar.dma_start(wrr[:], wrr_ext.ap())
        nc.scalar.dma_start(bk64[:], bk_ext.ap())
        nc.scalar.dma_start(bv64[:], bv_ext.ap())
        for k in range(T):
            nc.scalar.dma_start(wk[:, k, :], wk_ext.ap()[k * P : (k + 1) * P, :])
        for k in range(T):
            nc.scalar.dma_start(wv[:, k, :], wv_ext.ap()[k * P : (k + 1) * P, :])
        for k in range(T):
            rsl = slice(k * P, (k + 1) * P)
            nc.scalar.dma_start(xt8[:, k, : S // 2], xt8_ext.ap()[rsl, : S // 2])
            nc.scalar.dma_start(xt8[:, k, S // 2 :], xt8_ext.ap()[rsl, S // 2 :])

        def qreg_of(t, n):
            """q chunk n of tile t lives in xt chunk n-1's space (spare for
            n=0)."""
            src = qsp if n == 0 else xtc[n - 1]
            return src[:, t, :]

        # ---- Q phase: n-outer so xt chunks free up for q storage ----
        def q_tail(t, n):
            """Delayed-by-one-slot logit matmul + exp + pooled partials."""
            qreg = qreg_of(t, n)
            pl = psl.tile([P, NC_], F32, name="pl", tag="pl")
            nc.tensor.matmul(pl, wqlrep[:], qreg, start=True, stop=True)
            col = slice(t * NS + n, t * NS + n + 1)
            e = e_pool.tile([P, NC_], BF16, name="e", tag="e")
            nc.scalar.activation(
                e, pl, AF.Exp, bias=0.0, scale=1.0, accum_out=pe_q[:, col]
            )
            m = m_pool.tile([P, NC_], BF16, name="m", tag="m")
            if USE_TTR:
                nc.vector.tensor_tensor_reduce(
                    m, e, qreg, scale=INV_W8, scalar=0.0,
                    op0=ALU.mult, op1=ALU.add, accum_out=pq_q[:, col],
                )
            else:
                nc.vector.tensor_tensor(m, e, qreg, ALU.mult)
                nc.vector.reduce_sum(pq_q[:, col], m, axis=AX)

        pend = None
        for n in range(NS):
            for t in range(T):
                slot = n * T + t
                pch = psg.tile([P, NC_], F32, name="pg", tag="pg")
                for k in range(T):
                    nc.tensor.matmul(
                        pch, wq[:, k, t * P : (t + 1) * P], xtc[n][:, k, :],
                        start=(k == 0), stop=(k == T - 1),
                    )
                qreg = qreg_of(t, n)
                if slot % 2 == 0:
                    nc.scalar.activation(
                        qreg, pch, AF.Identity, bias=bqbr[:, t : t + 1], scale=1.0
                    )
                else:
                    nc.vector.tensor_scalar_add(qreg, pch, bqbr[:, t : t + 1])
                if pend is not None:
                    q_tail(*pend)
                pend = (t, n)
        q_tail(*pend)

        # Q pool finalizers: gq_all = gq_true/64 (br contribution removed).
        for t in range(T):
            tsl = slice(t * NS, (t + 1) * NS)
            stot = small_pool.tile([P, 1], F32, name="stot", tag="stot")
            nc.vector.reduce_sum(stot, pe_q[:, tsl], axis=AX)
            rec = small_pool.tile([P, 1], F32, name="rec", tag="rec")
            nc.vector.reciprocal(rec, stot)
            if not USE_TTR:
                nc.vector.tensor_scalar_mul(rec, rec, INV_W8)
            graw = small_pool.tile([P, 1], F32, name="graw", tag="graw")
            nc.vector.reduce_sum(graw, pq_q[:, tsl], axis=AX)
            tmp = small_pool.tile([P, 1], F32, name="gtmp", tag="gtmp")
            nc.vector.tensor_tensor(tmp, graw, rec, ALU.mult)
            nc.vector.tensor_tensor(gq_all[:, t : t + 1], tmp, br64[:], ALU.subtract)

        # ---- K phase: t-outer, per-chunk pipelined gated logits ----
        def k_tail(t, n, pt, eff):
            pl = psl.tile([P, NC_], F32, name="pl", tag="pl")
            nc.tensor.matmul(pl, eff[:], pt, start=True, stop=True)
            col = slice(t * NS + n, t * NS + n + 1)
            e = e_pool.tile([P, NC_], BF16, name="e", tag="e")
            nc.scalar.activation(
                e, pl, AF.Exp, bias=0.0, scale=1.0, accum_out=pe_k[:, col]
            )
            m = m_pool.tile([P, NC_], BF16, name="m", tag="m")
            if USE_TTR:
                nc.vector.tensor_tensor_reduce(
                    m, e, pt, scale=INV_W8, scalar=0.0,
                    op0=ALU.mult, op1=ALU.add, accum_out=pq_k[:, col],
                )
            else:
                nc.vector.tensor_tensor(m, e, pt, ALU.mult)
                nc.vector.reduce_sum(pq_k[:, col], m, axis=AX)

        def k_final(t):
            tsl = slice(t * NS, (t + 1) * NS)
            stot = small_pool.tile([P, 1], F32, name="stot", tag="stot")
            nc.vector.reduce_sum(stot, pe_k[:, tsl], axis=AX)
            rec = small_pool.tile([P, 1], F32, name="rec", tag="rec")
            nc.vector.reciprocal(rec, stot)
            if not USE_TTR:
                nc.vector.tensor_scalar_mul(rec, rec, INV_W8)
            graw = small_pool.tile([P, 1], F32, name="graw", tag="graw")
            nc.vector.reduce_sum(graw, pq_k[:, tsl], axis=AX)
            tmp = small_pool.tile([P, 1], F32, name="gtmp", tag="gtmp")
            nc.vector.tensor_tensor(tmp, graw, rec, ALU.mult)
            nc.vector.tensor_tensor(
                gk_all[:, t : t + 1], tmp, gq_all[:, t : t + 1], ALU.mult
            )

        kpend = None
        for t in range(T):
            eff = eff_pool.tile([P, P], BF16, name="effkl", tag="eff")
            nc.vector.tensor_scalar_mul(eff[:], wklrep[:], gq_all[:, t : t + 1])
            for n in range(NS):
                slot = t * NS + n
                pch = psg.tile([P, NC_], F32, name="pg", tag="pg")
                for kk in range(0, T, 2):
                    nc.tensor.matmul(
                        pch,
                        wk[:, kk : kk + 2, t * P : (t + 1) * P],
                        xt8[:, kk : kk + 2, n * NC_ : (n + 1) * NC_],
                        start=(kk == 0), stop=(kk == T - 2), perf_mode=DR,
                    )
                pt = chk_pool.tile([P, NC_], BF16, name="chk", tag="chk")
                if slot % 2 == 0:
                    nc.scalar.activation(
                        pt, pch, AF.Identity, bias=bk64[:, t : t + 1], scale=1.0
                    )
                else:
                    nc.vector.tensor_scalar_add(pt, pch, bk64[:, t : t + 1])
                if kpend is not None:
                    k_tail(*kpend)
                    if kpend[1] == NS - 1:
                        k_final(kpend[0])
                kpend = (t, n, pt, eff)
        k_tail(*kpend)
        k_final(T - 1)

        # ---- V phase: per-chunk rt matmul + residual add + store ----
        def v_tail(t, n, ut, eff):
            pl = psl.tile([P, NC_], F32, name="pl", tag="pl")
            nc.tensor.matmul(pl, eff[:], ut, start=True, stop=True)
            stg = stg_pool.tile([P, NC_], OUT_DT, name="stg", tag="stg")
            nc.vector.tensor_tensor(stg, pl, qreg_of(t, n), ALU.add)
            dma_eng = nc.sync if (t * NS + n) % 2 == 0 else nc.gpsimd
            dma_eng.dma_start(
                out_ext.ap()[t * P : (t + 1) * P, n * NC_ : (n + 1) * NC_], stg
            )

        vpend = None
        for t in range(T):
            eff = eff_pool.tile([P, P], BF16, name="effrt", tag="eff")
            nc.vector.tensor_scalar_mul(eff[:], wrr[:], gk_all[:, t : t + 1])
            for n in range(NS):
                slot = t * NS + n
                pch = psg.tile([P, NC_], F32, name="pg", tag="pg")
                for kk in range(0, T, 2):
                    nc.tensor.matmul(
                        pch,
                        wv[:, kk : kk + 2, t * P : (t + 1) * P],
                        xt8[:, kk : kk + 2, n * NC_ : (n + 1) * NC_],
                        start=(kk == 0), stop=(kk == T - 2), perf_mode=DR,
                    )
                ut = chk_pool.tile([P, NC_], BF16, name="chk", tag="chk")
                if slot % 2 == 0:
                    nc.scalar.activation(
                        ut, pch, AF.Identity, bias=bv64[:, t : t + 1], scale=1.0
                    )
                else:
                    nc.vector.tensor_scalar_add(ut, pch, bv64[:, t : t + 1])
                if vpend is not None:
                    v_tail(*vpend)
                vpend = (t, n, ut, eff)
        v_tail(*vpend)

    nc.compile()
    return nc


def _prep_shared(inputs):
    """Host-side prep of the replicated (weight) arrays."""
    sc = 0.125  # 1/sqrt(HD)

    def rep_logit(w):
        m = np.zeros((P, P), dtype=np.float32)
        ws = w.astype(np.float32) * sc
        m[:HD, :HD] = ws[:, None]          # rows d 0..63 -> head-0 columns
        m[HD:, HD:] = ws[:, None]          # rows d 64..127 -> head-1 columns
        return m.astype(BF)

    def bias_pp(b):
        return np.ascontiguousarray(b.astype(np.float32).reshape(T, P).T)

    wrr = np.zeros((P, P), dtype=np.float32)
    wr = inputs["Wr"].astype(np.float32)
    wrr[:HD, :HD] = wr
    wrr[HD:, HD:] = wr

    br_col = np.tile(inputs["br"].astype(np.float32), 2).reshape(P, 1)

    return {
        "wq": np.ascontiguousarray(inputs["Wq"].astype(BF)),
        "wk": np.ascontiguousarray(
            (inputs["Wk"].astype(np.float32) * W8SCALE).astype(F8)
        ),
        "wv": np.ascontiguousarray(
            (inputs["Wv"].astype(np.float32) * W8SCALE).astype(F8)
        ),
        "bqbr": bias_pp(inputs["bq"]) + br_col,
        "bk64": bias_pp(inputs["bk"]) * np.float32(W8SCALE),
        "bv64": bias_pp(inputs["bv"]) * np.float32(W8SCALE),
        "br64": np.ascontiguousarray(br_col * np.float32(INV_W8)),
        "wqlrep": rep_logit(inputs["wql"]),
        "wklrep": rep_logit(inputs["wkl"]),
        "wrr": wrr.astype(BF),
    }


def _get_nc():
    if "nc" not in _CACHE:
        _CACHE["nc"] = _build()
    return _CACHE["nc"]


def _run(inputs, trace=False):
    nc = _get_nc()
    shared = _prep_shared(inputs)
    X = inputs["X"]
    in_maps = []
    for b in range(N_CORES):
        m = dict(shared)
        xtb = np.ascontiguousarray(X[b].T)
        m["xt"] = xtb.astype(BF)
        m["xt8"] = xtb.astype(F8)
        in_maps.append(m)
    if trace:
        _install_profile_hook()
    res = run_bass_kernel_spmd(nc, in_maps, list(range(N_CORES)), trace=trace)
    out = np.empty((B, S, D), dtype=np.float32)
    for b in range(N_CORES):
        out[b] = np.asarray(res.results[b]["out"]).astype(np.float32).T
    return out, res


def _install_profile_hook():
    import antenv

    if "antenv.axon_hooks" not in sys.modules:
        mod = types.ModuleType("antenv.axon_hooks")
        mod._hook = None
        mod.set_axon_ntff_profile_hook = lambda h: setattr(mod, "_hook", h)
        mod.get_axon_ntff_profile_hook = lambda: mod._hook
        sys.modules["antenv.axon_hooks"] = mod
        antenv.axon_hooks = mod
    hooks = sys.modules["antenv.axon_hooks"]
    if hooks.get_axon_ntff_profile_hook() is None:
        from trn_agent_boot.trn_boot import _ntff_profile_via_ctypes

        hooks.set_axon_ntff_profile_hook(
            _ntff_profile_via_ctypes("/opt/axon/libaxon_pjrt.so")
        )
    import concourse.bass_utils as bass_utils

    bass_utils.upload_artifacts = lambda tmpdir: f"local:{tmpdir}"


def kernel(**inputs) -> np.ndarray:
    out, _ = _run(inputs, trace=False)
    return out


# revision 22
# speedup vs baseline: 1.0436x; 1.0436x over previous
"""Trainium2 Bass kernel for nn_AdditiveAttention (B=8, S=4096, D=1024, H=16).

Sharding: pure data-parallel over batch — 8 NeuronCores, one batch element
per core, weights replicated. No collectives.

v2 layout (everything transposed: d on partitions, s on free):
  - Q GEMM is n-outer (s-chunk outer, t-tile inner) so each xt s-chunk is
    dead right after its 8 output tiles are computed; q (bf16, +bq+br folded
    in) is written back into the xt chunk buffers with a one-chunk lag.
    Only one spare [128, 8, 512] buffer is needed for q chunk 0.
  - Per-chunk softmax pooling: logit matmul emitted one GEMM-slot late so
    PSUM evacuation always overlaps the next chunk's matmuls; exp+denominator
    fused on ScalarE (accum_out); numerator via one fused DVE
    tensor_tensor_reduce per chunk. No full-S e/p/u tiles anywhere.
  - K/V GEMMs in fp8 DoubleRow (weights host-scaled by 64; the 1/64 rides in
    the pooled-vector normalization), per-chunk gated logits / rt matmuls
    pipelined the same way.
  - Single bf16 output: out = q + (u @ Wr) (+bq+br already inside q),
    residual added during rt-PSUM evacuation on DVE. 8.4 MB written instead
    of the baseline's 33.6 MB f32 out+out2 pair.
  - wk/wv/xt8 prefetched on the scalar queue during the Q phase; startup
    loads are chunk-granular ([128,512]) and ordered so chunk 0 + wq arrive
    first on two issuing queues.
"""

import sys
import types

import numpy as np
import ml_dtypes

from contextlib import ExitStack

import concourse.bass as bass
import concourse.tile as tile
from concourse import bacc, mybir
from concourse.bass_utils import run_bass_kernel_spmd

B, S, D, H, HD = 8, 4096, 1024, 16, 64
P = 128          # partitions
T = D // P       # 8 d-tiles
NC_ = 512        # psum chunk free size
NS = S // NC_    # 8 s-chunks
N_CORES = 8
BF16 = mybir.dt.bfloat16
FP8 = mybir.dt.float8e4
F32 = mybir.dt.float32
W8SCALE = 64.0   # host scales Wk/Wv by this into e4m3 normal range
INV_W8 = 1.0 / W8SCALE
BF = ml_dtypes.bfloat16
F8 = ml_dtypes.float8_e4m3
OUT_DT = BF16  # bf16 halves output DMA traffic; host upcasts
# Pooled-sum (softmax numerator) implementation:
#   'stt_gpsimd': fused (e*(1/64))*src + accum via scalar_tensor_tensor on GpSimd
#   'stt_vector': same fused op on VectorE
#   'split':     tensor_tensor mult + reduce_sum, both on VectorE
POOL_MODE = "stt_vector"

_CACHE = {}


def _build():
    nc = bacc.Bacc(
        "TRN2", target_bir_lowering=False, debug=False, num_devices=N_CORES
    )
    xt_ext = nc.declare_dram_parameter("xt", [D, S], BF16, isOutput=False)
    xt8_ext = nc.declare_dram_parameter("xt8", [D, S], FP8, isOutput=False)
    wq_ext = nc.declare_dram_parameter("wq", [D, D], BF16, isOutput=False)
    wk_ext = nc.declare_dram_parameter("wk", [D, D], FP8, isOutput=False)
    wv_ext = nc.declare_dram_parameter("wv", [D, D], FP8, isOutput=False)
    bqbr_ext = nc.declare_dram_parameter("bqbr", [P, T], F32, isOutput=False)
    bk_ext = nc.declare_dram_parameter("bk64", [P, T], F32, isOutput=False)
    bv_ext = nc.declare_dram_parameter("bv64", [P, T], F32, isOutput=False)
    br64_ext = nc.declare_dram_parameter("br64", [P, 1], F32, isOutput=False)
    wql_ext = nc.declare_dram_parameter("wqlrep", [P, P], BF16, isOutput=False)
    wkl_ext = nc.declare_dram_parameter("wklrep", [P, P], BF16, isOutput=False)
    wrr_ext = nc.declare_dram_parameter("wrr", [P, P], BF16, isOutput=False)
    out_ext = nc.declare_dram_parameter("out", [D, S], OUT_DT, isOutput=True)

    AX = mybir.AxisListType.X
    ALU = mybir.AluOpType
    AF = mybir.ActivationFunctionType
    DR = mybir.MatmulPerfMode.DoubleRow

    with tile.TileContext(nc) as tc, ExitStack() as ctx:
        singles = ctx.enter_context(tc.tile_pool(name="singles", bufs=1))
        psg = ctx.enter_context(tc.tile_pool(name="psg", bufs=3, space="PSUM"))
        psl = ctx.enter_context(tc.tile_pool(name="psl", bufs=3, space="PSUM"))
        chk_pool = ctx.enter_context(tc.tile_pool(name="chk", bufs=3))
        e_pool = ctx.enter_context(tc.tile_pool(name="epool", bufs=2))
        m_pool = ctx.enter_context(tc.tile_pool(name="mpool", bufs=2))
        stg_pool = ctx.enter_context(tc.tile_pool(name="stg", bufs=3))
        eff_pool = ctx.enter_context(tc.tile_pool(name="eff", bufs=2))
        small_pool = ctx.enter_context(tc.tile_pool(name="small", bufs=2))

        # ---- resident tiles ----
        xtc = [
            singles.tile([P, T, NC_], BF16, name=f"xtc{n}", tag=f"xtc{n}")
            for n in range(NS)
        ]
        qsp = singles.tile([P, T, NC_], BF16, name="qsp", tag="qsp")
        xt8 = singles.tile([P, T, S], FP8, name="xt8", tag="xt8")
        wq = singles.tile([P, T, D], BF16, name="wq", tag="wq")
        wk = singles.tile([P, T, D], FP8, name="wk", tag="wk")
        wv = singles.tile([P, T, D], FP8, name="wv", tag="wv")
        wqlrep = singles.tile([P, P], BF16, name="wqlrep", tag="wqlrep")
        wklrep = singles.tile([P, P], BF16, name="wklrep", tag="wklrep")
        wrr = singles.tile([P, P], BF16, name="wrr", tag="wrr")
        bqbr = singles.tile([P, T], F32, name="bqbr", tag="bqbr")
        bk64 = singles.tile([P, T], F32, name="bk64", tag="bk64")
        bv64 = singles.tile([P, T], F32, name="bv64", tag="bv64")
        br64 = singles.tile([P, 1], F32, name="br64", tag="br64")
        gq_all = singles.tile([P, T], F32, name="gq", tag="gq")
        gk_all = singles.tile([P, T], F32, name="gk", tag="gk")
        pe_q = singles.tile([P, T * NS], F32, name="peq", tag="peq")
        pq_q = singles.tile([P, T * NS], F32, name="pqq", tag="pqq")
        pe_k = singles.tile([P, T * NS], F32, name="pek", tag="pek")
        pq_k = singles.tile([P, T * NS], F32, name="pqk", tag="pqk")

        # ---- DMA issue (ordering matters for startup) ----
        # sync: xt chunks 0,2,4,6 (chunk 0 first: the first GEMM chunk).
        for n in (0, 2, 4, 6):
            for k in range(T):
                nc.sync.dma_start(
                    xtc[n][:, k, :],
                    xt_ext.ap()[k * P : (k + 1) * P, n * NC_ : (n + 1) * NC_],
                )
        # gpsimd: wq first (needed by every chunk), then odd xt chunks.
        for k in range(T):
            nc.gpsimd.dma_start(wq[:, k, :], wq_ext.ap()[k * P : (k + 1) * P, :])
        for n in (1, 3, 5, 7):
            for k in range(T):
                nc.gpsimd.dma_start(
                    xtc[n][:, k, :],
                    xt_ext.ap()[k * P : (k + 1) * P, n * NC_ : (n + 1) * NC_],
                )
        # scalar: small weights/biases, then K/V weights + fp8 X (all needed
        # only from the K phase on).
        nc.scalar.dma_start(wqlrep[:], wql_ext.ap())
        nc.scalar.dma_start(bqbr[:], bqbr_ext.ap())
        nc.scalar.dma_start(br64[:], br64_ext.ap())
        nc.scalar.dma_start(wklrep[:], wkl_ext.ap())
        nc.scalar.dma_start(wrr[:], wrr_ext.ap())
        nc.scalar.dma_start(bk64[:], bk_ext.ap())
        nc.scalar.dma_start(bv64[:], bv_ext.ap())

        def kv_prefetch(n):
            """K/V-phase loads, paced: issued on the scalar queue at chunk-n
            boundaries of the Q loop so they don't contend with the Q-phase
            chunk streaming that feeds TensorE."""
            if n == 1:
                for k in range(T):
                    nc.scalar.dma_start(
                        wk[:, k, :], wk_ext.ap()[k * P : (k + 1) * P, :]
                    )
            elif n == 2:
                for k in range(T):
                    nc.scalar.dma_start(
                        wv[:, k, :], wv_ext.ap()[k * P : (k + 1) * P, :]
                    )
            elif 3 <= n <= 6:
                for k in (2 * (n - 3), 2 * (n - 3) + 1):
                    rsl = slice(k * P, (k + 1) * P)
                    nc.scalar.dma_start(
                        xt8[:, k, : S // 2], xt8_ext.ap()[rsl, : S // 2]
                    )
                    nc.scalar.dma_start(
                        xt8[:, k, S // 2 :], xt8_ext.ap()[rsl, S // 2 :]
                    )

        def qreg_of(t, n):
            """q chunk n of tile t lives in xt chunk n-1's space (spare for
            n=0)."""
            src = qsp if n == 0 else xtc[n - 1]
            return src[:, t, :]

        def pool_sum(e, src, accum_col):
            """accum_col = sum_s e[:,s]*src[:,s] / 64 (the 1/64 un-scales the
            fp8 K/V weight scaling; for Q it cancels in num/denom)."""
            m = m_pool.tile([P, NC_], BF16, name="m", tag="m")
            if POOL_MODE == "stt_gpsimd":
                nc.gpsimd.scalar_tensor_tensor(
                    m, e, INV_W8, src, op0=ALU.mult, op1=ALU.mult,
                    accum_out=accum_col,
                )
            elif POOL_MODE == "stt_vector":
                nc.vector.scalar_tensor_tensor(
                    m, e, INV_W8, src, op0=ALU.mult, op1=ALU.mult,
                    accum_out=accum_col,
                )
            else:
                nc.vector.tensor_tensor(m, e, src, ALU.mult)
                nc.vector.reduce_sum(accum_col, m, axis=AX)

        # ---- Q phase: n-outer so xt chunks free up for q storage ----
        def q_tail(t, n):
            """Delayed-by-one-slot logit matmul + exp + pooled partials."""
            qreg = qreg_of(t, n)
            pl = psl.tile([P, NC_], F32, name="pl", tag="pl")
            nc.tensor.matmul(pl, wqlrep[:], qreg, start=True, stop=True)
            col = slice(t * NS + n, t * NS + n + 1)
            e = e_pool.tile([P, NC_], BF16, name="e", tag="e")
            nc.scalar.activation(
                e, pl, AF.Exp, bias=0.0, scale=1.0, accum_out=pe_q[:, col]
            )
            pool_sum(e, qreg, pq_q[:, col])

        pend = None
        for n in range(NS):
            for t in range(T):
                slot = n * T + t
                pch = psg.tile([P, NC_], F32, name="pg", tag="pg")
                for k in range(T):
                    nc.tensor.matmul(
                        pch, wq[:, k, t * P : (t + 1) * P], xtc[n][:, k, :],
                        start=(k == 0), stop=(k == T - 1),
                    )
                qreg = qreg_of(t, n)
                if slot % 2 == 0:
                    nc.scalar.activation(
                        qreg, pch, AF.Identity, bias=bqbr[:, t : t + 1], scale=1.0
                    )
                else:
                    nc.vector.tensor_scalar_add(qreg, pch, bqbr[:, t : t + 1])
                if pend is not None:
                    q_tail(*pend)
                pend = (t, n)
            kv_prefetch(n)
        q_tail(*pend)

        # Q pool finalizers: gq_all = gq_true/64 (br contribution removed).
        for t in range(T):
            tsl = slice(t * NS, (t + 1) * NS)
            stot = small_pool.tile([P, 1], F32, name="stot", tag="stot")
            nc.vector.reduce_sum(stot, pe_q[:, tsl], axis=AX)
            rec = small_pool.tile([P, 1], F32, name="rec", tag="rec")
            nc.vector.reciprocal(rec, stot)
            if POOL_MODE == "split":
                nc.vector.tensor_scalar_mul(rec, rec, INV_W8)
            graw = small_pool.tile([P, 1], F32, name="graw", tag="graw")
            nc.vector.reduce_sum(graw, pq_q[:, tsl], axis=AX)
            tmp = small_pool.tile([P, 1], F32, name="gtmp", tag="gtmp")
            nc.vector.tensor_tensor(tmp, graw, rec, ALU.mult)
            nc.vector.tensor_tensor(gq_all[:, t : t + 1], tmp, br64[:], ALU.subtract)

        # ---- K phase: t-outer, per-chunk pipelined gated logits ----
        def k_tail(t, n, pt, eff):
            pl = psl.tile([P, NC_], F32, name="pl", tag="pl")
            nc.tensor.matmul(pl, eff[:], pt, start=True, stop=True)
            col = slice(t * NS + n, t * NS + n + 1)
            e = e_pool.tile([P, NC_], BF16, name="e", tag="e")
            nc.scalar.activation(
                e, pl, AF.Exp, bias=0.0, scale=1.0, accum_out=pe_k[:, col]
            )
            pool_sum(e, pt, pq_k[:, col])

        def k_final(t):
            tsl = slice(t * NS, (t + 1) * NS)
            stot = small_pool.tile([P, 1], F32, name="stot", tag="stot")
            nc.vector.reduce_sum(stot, pe_k[:, tsl], axis=AX)
            rec = small_pool.tile([P, 1], F32, name="rec", tag="rec")
            nc.vector.reciprocal(rec, stot)
            if POOL_MODE == "split":
                nc.vector.tensor_scalar_mul(rec, rec, INV_W8)
            graw = small_pool.tile([P, 1], F32, name="graw", tag="graw")
            nc.vector.reduce_sum(graw, pq_k[:, tsl], axis=AX)
            tmp = small_pool.tile([P, 1], F32, name="gtmp", tag="gtmp")
            nc.vector.tensor_tensor(tmp, graw, rec, ALU.mult)
            nc.vector.tensor_tensor(
                gk_all[:, t : t + 1], tmp, gq_all[:, t : t + 1], ALU.mult
            )

        kpend = None
        for t in range(T):
            eff = eff_pool.tile([P, P], BF16, name="effkl", tag="eff")
            nc.vector.tensor_scalar_mul(eff[:], wklrep[:], gq_all[:, t : t + 1])
            for n in range(NS):
                slot = t * NS + n
                pch = psg.tile([P, NC_], F32, name="pg", tag="pg")
                for kk in range(0, T, 2):
                    nc.tensor.matmul(
                        pch,
                        wk[:, kk : kk + 2, t * P : (t + 1) * P],
                        xt8[:, kk : kk + 2, n * NC_ : (n + 1) * NC_],
                        start=(kk == 0), stop=(kk == T - 2), perf_mode=DR,
                    )
                pt = chk_pool.tile([P, NC_], BF16, name="chk", tag="chk")
                nc.vector.tensor_scalar_add(pt, pch, bk64[:, t : t + 1])
                if kpend is not None:
                    k_tail(*kpend)
                    if kpend[1] == NS - 1:
                        k_final(kpend[0])
                kpend = (t, n, pt, eff)
        k_tail(*kpend)
        k_final(T - 1)

        # ---- V phase: per-chunk rt matmul + residual add + store ----
        def v_tail(t, n, ut, eff):
            pl = psl.tile([P, NC_], F32, name="pl", tag="pl")
            nc.tensor.matmul(pl, eff[:], ut, start=True, stop=True)
            stg = stg_pool.tile([P, NC_], OUT_DT, name="stg", tag="stg")
            nc.vector.tensor_tensor(stg, pl, qreg_of(t, n), ALU.add)
            dma_eng = nc.sync if (t * NS + n) % 2 == 0 else nc.gpsimd
            dma_eng.dma_start(
                out_ext.ap()[t * P : (t + 1) * P, n * NC_ : (n + 1) * NC_], stg
            )

        vpend = None
        for t in range(T):
            eff = eff_pool.tile([P, P], BF16, name="effrt", tag="eff")
            nc.vector.tensor_scalar_mul(eff[:], wrr[:], gk_all[:, t : t + 1])
            for n in range(NS):
                slot = t * NS + n
                pch = psg.tile([P, NC_], F32, name="pg", tag="pg")
                for kk in range(0, T, 2):
                    nc.tensor.matmul(
                        pch,
                        wv[:, kk : kk + 2, t * P : (t + 1) * P],
                        xt8[:, kk : kk + 2, n * NC_ : (n + 1) * NC_],
                        start=(kk == 0), stop=(kk == T - 2), perf_mode=DR,
                    )
                ut = chk_pool.tile([P, NC_], BF16, name="chk", tag="chk")
                if slot % 2 == 0:
                    nc.scalar.activation(
                        ut, pch, AF.Identity, bias=bv64[:, t : t + 1], scale=1.0
                    )
                else:
                    nc.vector.tensor_scalar_add(ut, pch, bv64[:, t : t + 1])
                if vpend is not None:
                    v_tail(*vpend)
                vpend = (t, n, ut, eff)
        v_tail(*vpend)

    nc.compile()
    return nc


def _prep_shared(inputs):
    """Host-side prep of the replicated (weight) arrays."""
    sc = 0.125  # 1/sqrt(HD)

    def rep_logit(w):
        m = np.zeros((P, P), dtype=np.float32)
        ws = w.astype(np.float32) * sc
        m[:HD, :HD] = ws[:, None]          # rows d 0..63 -> head-0 columns
        m[HD:, HD:] = ws[:, None]          # rows d 64..127 -> head-1 columns
        return m.astype(BF)

    def bias_pp(b):
        return np.ascontiguousarray(b.astype(np.float32).reshape(T, P).T)

    wrr = np.zeros((P, P), dtype=np.float32)
    wr = inputs["Wr"].astype(np.float32)
    wrr[:HD, :HD] = wr
    wrr[HD:, HD:] = wr

    br_col = np.tile(inputs["br"].astype(np.float32), 2).reshape(P, 1)

    return {
        "wq": np.ascontiguousarray(inputs["Wq"].astype(BF)),
        "wk": np.ascontiguousarray(
            (inputs["Wk"].astype(np.float32) * W8SCALE).astype(F8)
        ),
        "wv": np.ascontiguousarray(
            (inputs["Wv"].astype(np.float32) * W8SCALE).astype(F8)
        ),
        "bqbr": bias_pp(inputs["bq"]) + br_col,
        "bk64": bias_pp(inputs["bk"]) * np.float32(W8SCALE),
        "bv64": bias_pp(inputs["bv"]) * np.float32(W8SCALE),
        "br64": np.ascontiguousarray(br_col * np.float32(INV_W8)),
        "wqlrep": rep_logit(inputs["wql"]),
        "wklrep": rep_logit(inputs["wkl"]),
        "wrr": wrr.astype(BF),
    }


def _get_nc():
    if "nc" not in _CACHE:
        _CACHE["nc"] = _build()
    return _CACHE["nc"]


def _run(inputs, trace=False):
    nc = _get_nc()
    shared = _prep_shared(inputs)
    X = inputs["X"]
    in_maps = []
    for b in range(N_CORES):
        m = dict(shared)
        xtb = np.ascontiguousarray(X[b].T)
        m["xt"] = xtb.astype(BF)
        m["xt8"] = xtb.astype(F8)
        in_maps.append(m)
    if trace:
        _install_profile_hook()
    res = run_bass_kernel_spmd(nc, in_maps, list(range(N_CORES)), trace=trace)
    out = np.empty((B, S, D), dtype=np.float32)
    for b in range(N_CORES):
        out[b] = np.asarray(res.results[b]["out"]).astype(np.float32).T
    return out, res


def _install_profile_hook():
    import antenv

    if "antenv.axon_hooks" not in sys.modules:
        mod = types.ModuleType("antenv.axon_hooks")
        mod._hook = None
        mod.set_axon_ntff_profile_hook = lambda h: setattr(mod, "_hook", h)
        mod.get_axon_ntff_profile_hook = lambda: mod._hook
        sys.modules["antenv.axon_hooks"] = mod
        antenv.axon_hooks = mod
    hooks = sys.modules["antenv.axon_hooks"]
    if hooks.get_axon_ntff_profile_hook() is None:
        from trn_agent_boot.trn_boot import _ntff_profile_via_ctypes

        hooks.set_axon_ntff_profile_hook(
            _ntff_profile_via_ctypes("/opt/axon/libaxon_pjrt.so")
        )
    import concourse.bass_utils as bass_utils

    bass_utils.upload_artifacts = lambda tmpdir: f"local:{tmpdir}"


def kernel(**inputs) -> np.ndarray:
    out, _ = _run(inputs, trace=False)
    return out
